# revision 8
# baseline (speedup 1.0000x reference)
"""Trainium2 Bass kernel for nn_MlpNet: Gaussian-window spectrogram + MLP.

Math:
  s[b, f, t] = |sum_n xc[b,n] * win[t,n] * e^{-2pi i f n / 2046}|^2
  out = relu(s.flat @ W1.T + b1) @ W2.T + b2

Sharding (8 cores): frequency bins f are split 128-per-core. Each core
computes its s[b, f_shard, t] slab via two f32r matmuls (cos/sin DFT
against the windowed signal), squares+adds on ACT/DVE, and accumulates
its fc1 partial h (bf16 matmuls against its column shard of W1, whose
feature index f*T+t is contiguous per f-shard). The host sums the 8
h-partials, applies b1/relu/fc2, and concatenates the s slabs.

The Gaussian window is truncated at |n - t| > 8 sigma (below the fp32
noise floor of the 1023-term sum), which skips ~40% of the DFT matmuls.
"""
import os
import sys

sys.path.insert(0, "/opt/trn_rl_repo")

import numpy as np
import ml_dtypes

B = 16
N = 1023
T = 1024
F = 1024
HID = 128
NCLS = 10
NCORES = 8
FPC = F // NCORES       # 128 frequency bins per core
KT = 8                  # contraction tiles of 128 over n
TBS = 256               # t-block size (matmul free dim)
NTB = T // TBS          # 4 t-blocks
GS = 64                 # fc1 chunks per W1 DMA slab
NSLAB = T // GS         # 16 slabs

LAST_RESULT = None
LAST_TIMES = []


def _make_blocks(band):
    """Flatten the (tb, k) band structure into a packed block list;
    returns (blocks, index) where blocks[i] = (k, tb) and
    index[(k, tb)] = i."""
    blocks = []
    index = {}
    for tb in range(NTB):
        for k in band[tb]:
            index[(k, tb)] = len(blocks)
            blocks.append((k, tb))
    return blocks, index


def _make_band(sigma):
    """Per t-block list of n-tiles (of 128) where the Gaussian window is
    non-negligible: |n - t| <= ceil(8 sigma) keeps every term above the
    fp32 noise floor of the sum."""
    w = int(np.ceil(8.0 * max(sigma, 1.0)))
    band = []
    for tb in range(NTB):
        lo = tb * TBS - w
        hi = tb * TBS + TBS - 1 + w
        ks = [k for k in range(KT) if 128 * k <= hi and 128 * k + 127 >= lo]
        band.append(ks)
    return band


def _build_program(band, nblk):
    import concourse.bacc as bacc
    import concourse.tile as tile
    from concourse import mybir

    f32 = mybir.dt.float32
    f32r = mybir.dt.float32r
    bf16 = mybir.dt.bfloat16
    Square = mybir.ActivationFunctionType.Square

    nc = bacc.Bacc("TRN2", target_bir_lowering=False, debug=False,
                   num_devices=NCORES)

    xct_d = nc.dram_tensor("xct", [128, KT * B], f32, kind="ExternalInput")
    wint_d = nc.dram_tensor("wint", [128, nblk * TBS], f32r, kind="ExternalInput")
    cs_d = nc.dram_tensor("cs", [128, KT * 2 * FPC], f32, kind="ExternalInput")
    w1h_d = nc.dram_tensor("w1h", [128, T * HID], bf16, kind="ExternalInput")
    s_out_d = nc.dram_tensor("s_out", [B, FPC, T], f32, kind="ExternalOutput")
    h_out_d = nc.dram_tensor("h_out", [B, HID], f32, kind="ExternalOutput")

    blocks, blkidx = _make_blocks(band)
    with tile.TileContext(nc) as tc:
        with (
            tc.tile_pool(name="const", bufs=1) as const_pool,
            tc.tile_pool(name="sbf", bufs=1) as sbf_pool,
            tc.tile_pool(name="csb", bufs=2) as csb_pool,
            tc.tile_pool(name="sq", bufs=4) as sq_pool,
            tc.tile_pool(name="stage", bufs=2) as stage_pool,
            tc.tile_pool(name="w1", bufs=6) as w1_pool,
            tc.tile_pool(name="hacc", bufs=1) as hacc_pool,
            tc.tile_pool(name="ps", bufs=2, space="PSUM") as ps_pool,
            tc.tile_pool(name="hps", bufs=1, space="PSUM") as hps_pool,
        ):
            # All input loads go on the sync HWDGE ring: FIFO order
            # guarantees the small consts land before the W1 stream starts.
            xct_sb = const_pool.tile([128, KT * B], f32)
            wint_sb = const_pool.tile([128, nblk * TBS], f32r)
            cs_sb = const_pool.tile([128, KT * 2 * FPC], f32)
            nc.sync.dma_start(xct_sb[:], xct_d[:])
            nc.sync.dma_start(cs_sb[:], cs_d[:])
            half = (nblk // 2) * TBS
            nc.sync.dma_start(wint_sb[:, :half], wint_d[:, :half])
            nc.sync.dma_start(wint_sb[:, half:], wint_d[:, half:])

            # bf16 copy of this core's s slab, laid out [f_p, b*T + t],
            # read back as the fc1 stationary operand.
            s_bf = sbf_pool.tile([128, B * T], bf16)
            s_bf_r = s_bf[:].rearrange("p (b t) -> p t b", b=B)

            # Half-split schedule: spectrogram for t-halves, with fc1 for
            # half h interleaved after its spectro so the W1 stream starts
            # draining mid-kernel instead of all at the end.
            h_acc = hacc_pool.tile([B, HID], f32)
            nc.vector.memset(h_acc[:], 0.0)

            HNTB = NTB // 2
            for half in range(2):
                tbs_here = range(half * HNTB, (half + 1) * HNTB)
                # ---- spectrogram: per-sample DFT of the windowed signal ----
                for b in range(B):
                    csb = csb_pool.tile([128, KT * 2 * FPC], f32r)
                    kset = sorted({k for tb in tbs_here for k in band[tb]})
                    for k in kset:
                        nc.vector.tensor_scalar_mul(
                            csb[:, k * 256:(k + 1) * 256],
                            cs_sb[:, k * 256:(k + 1) * 256],
                            xct_sb[:, k * B + b:k * B + b + 1],
                        )
                    s_stage = stage_pool.tile([128, T // 2], f32)
                    for tb in tbs_here:
                        t0 = tb * TBS
                        st0 = (tb - half * HNTB) * TBS
                        ks = band[tb]
                        ps_re = ps_pool.tile([128, TBS], f32, tag="psre")
                        ps_im = ps_pool.tile([128, TBS], f32, tag="psim")
                        for i, k in enumerate(ks):
                            blk = blkidx[(k, tb)]
                            rhs = wint_sb[:, blk * TBS:(blk + 1) * TBS]
                            nc.tensor.matmul(
                                ps_re[:], csb[:, k * 256:k * 256 + 128], rhs,
                                start=(i == 0), stop=(i == len(ks) - 1),
                            )
                            nc.tensor.matmul(
                                ps_im[:], csb[:, k * 256 + 128:(k + 1) * 256],
                                rhs,
                                start=(i == 0), stop=(i == len(ks) - 1),
                            )
                        t_re = sq_pool.tile([128, TBS], f32, tag="tre")
                        t_im = sq_pool.tile([128, TBS], f32, tag="tim")
                        nc.scalar.activation(t_re[:], ps_re[:], Square)
                        nc.scalar.activation(t_im[:], ps_im[:], Square)
                        nc.vector.tensor_add(
                            s_stage[:, st0:st0 + TBS], t_re[:], t_im[:]
                        )
                        nc.vector.tensor_copy(
                            s_bf[:, b * T + t0:b * T + t0 + TBS],
                            s_stage[:, st0:st0 + TBS],
                        )
                    nc.gpsimd.dma_start(
                        s_out_d[b, :, half * (T // 2):(half + 1) * (T // 2)],
                        s_stage[:],
                    )

                # ---- fc1 partial for this half's t-chunks ----
                h_ps = hps_pool.tile([B, HID], f32)
                t_lo = half * (T // 2)
                t_hi = (half + 1) * (T // 2)
                for sl in range(t_lo // GS, t_hi // GS):
                    c0 = sl * GS * HID
                    w1t = w1_pool.tile([128, GS * HID], bf16)
                    nc.sync.dma_start(w1t[:], w1h_d[:, c0:c0 + GS * HID])
                    for g in range(GS):
                        t = sl * GS + g
                        nc.tensor.matmul(
                            h_ps[:],
                            s_bf_r[:, t, :],
                            w1t[:, g * HID:(g + 1) * HID],
                            start=(t == t_lo),
                            stop=(t == t_hi - 1),
                        )
                nc.vector.tensor_add(h_acc[:], h_acc[:], h_ps[:])
            nc.gpsimd.dma_start(h_out_d[:], h_acc[:])

    nc.compile()
    return nc


def _host_precompute(x, lambd):
    sigma = float(abs(np.float32(np.asarray(lambd))))
    n_idx = np.arange(1024, dtype=np.float64)[:, None]   # padded n (1023 -> 0)
    t_idx = np.arange(T, dtype=np.float64)[None, :]

    xc = (x - x.mean(axis=-1, keepdims=True)).astype(np.float32)  # [B, N]
    xct = np.zeros((1024, B), np.float32)
    xct[:N] = xc.T
    xct = xct.reshape(KT, 128, B).transpose(1, 0, 2).reshape(128, KT * B)
    xct = np.ascontiguousarray(xct)

    winT = np.exp(-0.5 * ((n_idx - t_idx) / sigma) ** 2).astype(np.float32)
    winT[N:] = 0.0
    wfull = winT.reshape(KT, 128, T)      # [k, p, t]

    cs_list = []
    for c in range(NCORES):
        f_idx = (c * FPC + np.arange(FPC, dtype=np.float64))[None, :]
        ang = 2.0 * np.pi * n_idx * f_idx / (2.0 * N)
        cs = np.concatenate(
            [np.cos(ang), np.sin(ang)], axis=1
        ).astype(np.float32)                                  # [1024, 256]
        cs[N:] = 0.0
        cs = cs.reshape(KT, 128, 2 * FPC).transpose(1, 0, 2)
        cs_list.append(np.ascontiguousarray(cs.reshape(128, KT * 2 * FPC)))

    return sigma, xct, wfull, cs_list


def _transpose_w1(W1):
    """Blocked transpose W1 [HID, F*T] -> [F*T, HID] in bf16."""
    feat = W1.shape[1]
    W1T = np.empty((feat, HID), ml_dtypes.bfloat16)
    bs = 8192
    for i in range(0, feat, bs):
        W1T[i:i + bs] = W1[:, i:i + bs].T.astype(ml_dtypes.bfloat16)
    return W1T


def kernel(x, lambd, W1, b1, W2, b2):
    global LAST_RESULT
    from concourse.bass_utils import run_bass_kernel_spmd

    x = np.asarray(x, np.float32)
    W1 = np.asarray(W1, np.float32)
    b1 = np.asarray(b1, np.float32)
    W2 = np.asarray(W2, np.float32)
    b2 = np.asarray(b2, np.float32)

    sigma, xct, wfull, cs_list = _host_precompute(x, lambd)
    band = _make_band(sigma)
    blocks, _ = _make_blocks(band)
    wint = np.concatenate(
        [wfull[k][:, tb * TBS:(tb + 1) * TBS] for (k, tb) in blocks], axis=1
    )
    wint = np.ascontiguousarray(wint)

    W1T = _transpose_w1(W1)                     # [F*T, HID] bf16
    w1h = W1T.reshape(NCORES, 128, T * HID)     # per-core contiguous views

    nc = _build_program(band, len(blocks))

    in_maps = [
        dict(xct=xct, wint=wint, cs=cs_list[c], w1h=np.ascontiguousarray(w1h[c]))
        for c in range(NCORES)
    ]
    repeat = int(os.environ.get("KERNEL_REPEAT", "1"))
    trace = bool(os.environ.get("KERNEL_TRACE"))
    times = []
    res = None
    for _ in range(repeat):
        res = run_bass_kernel_spmd(nc, in_maps, list(range(NCORES)), trace=trace)
        if res.exec_time_ns is not None:
            times.append(res.exec_time_ns)
    LAST_RESULT = res
    global LAST_TIMES
    LAST_TIMES = times

    s = np.concatenate(
        [res.results[c]["s_out"] for c in range(NCORES)], axis=1
    )                                            # [B, F, T]
    h = np.zeros((B, HID), np.float32)
    for c in range(NCORES):
        h += res.results[c]["h_out"]
    h = np.maximum(h + b1, 0.0)
    out = h @ W2.T + b2
    return out.astype(np.float32), s[:, None].astype(np.float32)


# revision 20
# speedup vs baseline: 1.3947x; 1.3947x over previous
"""Trainium2 Bass kernel for nn_MlpNet: Gaussian-window spectrogram + MLP.

Math:
  s[b, f, t] = |sum_n xc[b,n] * win[t,n] * e^{-2pi i f n / 2046}|^2
  out = relu(s.flat @ W1.T + b1) @ W2.T + b2

Sharding (8 cores): frequency bins f are split 128-per-core. Each core
computes its s[b, f_shard, t] slab via two f32r matmuls (cos/sin DFT
against the windowed signal), squares+adds on ACT/DVE, and accumulates
its fc1 partial h (bf16 matmuls against its column shard of W1, whose
feature index f*T+t is contiguous per f-shard). The host sums the 8
h-partials, applies b1/relu/fc2, and concatenates the s slabs.

The Gaussian window is truncated at |n - t| > 8 sigma (below the fp32
noise floor of the 1023-term sum), which skips ~40% of the DFT matmuls.
"""
import os
import sys

sys.path.insert(0, "/opt/trn_rl_repo")

import numpy as np
import ml_dtypes

B = 16
N = 1023
T = 1024
F = 1024
HID = 128
NCLS = 10
NCORES = 8
FPC = F // NCORES       # 128 frequency bins per core
KT = 8                  # contraction tiles of 128 over n
TBS = 256               # t-block size (matmul free dim)
NTB = T // TBS          # 4 t-blocks
GS = 64                 # fc1 chunks per W1 DMA slab
NSLAB = T // GS         # 16 slabs

LAST_RESULT = None
LAST_TIMES = []


def _make_blocks(band):
    """Flatten the (tb, k) band structure into a packed block list;
    returns (blocks, index) where blocks[i] = (k, tb) and
    index[(k, tb)] = i."""
    blocks = []
    index = {}
    for tb in range(NTB):
        for k in band[tb]:
            index[(k, tb)] = len(blocks)
            blocks.append((k, tb))
    return blocks, index


def _make_band(sigma):
    """Per t-block list of n-tiles (of 128) where the Gaussian window is
    non-negligible: |n - t| <= ceil(8 sigma) keeps every term above the
    fp32 noise floor of the sum."""
    w = int(np.ceil(8.0 * max(sigma, 1.0)))
    band = []
    for tb in range(NTB):
        lo = tb * TBS - w
        hi = tb * TBS + TBS - 1 + w
        ks = [k for k in range(KT) if 128 * k <= hi and 128 * k + 127 >= lo]
        band.append(ks)
    return band


def _build_program(band, nblk):
    import concourse.bacc as bacc
    import concourse.tile as tile
    from concourse import mybir

    f32 = mybir.dt.float32
    f16 = mybir.dt.float16
    Square = mybir.ActivationFunctionType.Square

    nc = bacc.Bacc("TRN2", target_bir_lowering=False, debug=False,
                   num_devices=NCORES)

    xct_d = nc.dram_tensor("xct", [128, KT * B], f32, kind="ExternalInput")
    wint_d = nc.dram_tensor("wint", [128, nblk * TBS], f16, kind="ExternalInput")
    cs_d = nc.dram_tensor("cs", [128, KT * 2 * FPC], f32, kind="ExternalInput")
    w1h_d = nc.dram_tensor("w1h", [128, T * HID], f16, kind="ExternalInput")
    s_out_d = nc.dram_tensor("s_out", [B, FPC, T], f32, kind="ExternalOutput")
    h_out_d = nc.dram_tensor("h_out", [B, HID], f32, kind="ExternalOutput")

    blocks, blkidx = _make_blocks(band)
    with tile.TileContext(nc) as tc:
        with (
            tc.tile_pool(name="const", bufs=1) as const_pool,
            tc.tile_pool(name="sbf", bufs=1) as sbf_pool,
            tc.tile_pool(name="csb", bufs=1) as csb_pool,
            tc.tile_pool(name="sq", bufs=3) as sq_pool,
            tc.tile_pool(name="stage", bufs=1) as stage_pool,
            tc.tile_pool(name="w1", bufs=4) as w1_pool,
            tc.tile_pool(name="hacc", bufs=1) as hacc_pool,
            tc.tile_pool(name="ps", bufs=2, space="PSUM") as ps_pool,
            tc.tile_pool(name="hps", bufs=1, space="PSUM") as hps_pool,
        ):
            # All input loads go on the sync HWDGE ring: FIFO order
            # guarantees the small consts land before the W1 stream starts.
            xct_sb = const_pool.tile([128, KT * B], f32)
            wint_sb = const_pool.tile([128, nblk * TBS], f16)
            cs_sb = const_pool.tile([128, KT * 2 * FPC], f32)
            nc.sync.dma_start(xct_sb[:], xct_d[:])
            nc.sync.dma_start(cs_sb[:, :1024], cs_d[:, :1024])
            nc.sync.dma_start(wint_sb[:], wint_d[:])
            nc.sync.dma_start(cs_sb[:, 1024:], cs_d[:, 1024:])

            # fp16 copy of this core's s slab, laid out [f_p, b*T + t],
            # read back as the fc1 stationary operand.
            s_bf = sbf_pool.tile([128, B * T], f16)
            s_bf_r = s_bf[:].rearrange("p (b t) -> p t b", b=B)

            h_acc = hacc_pool.tile([B, HID], f32)
            nc.vector.memset(h_acc[:], 0.0)

            # Per-sample DFT matrices (cos|sin scaled by the centered
            # signal), cached for all B samples in fp16.
            def build_csb(csb, b, ks):
                for i, k in enumerate(ks):
                    if len(ks) > 2 and i == len(ks) - 1:
                        nc.scalar.activation(
                            csb[:, k * 256:(k + 1) * 256],
                            cs_sb[:, k * 256:(k + 1) * 256],
                            mybir.ActivationFunctionType.Copy,
                            scale=xct_sb[:, k * B + b:k * B + b + 1],
                        )
                    else:
                        nc.vector.tensor_scalar_mul(
                            csb[:, k * 256:(k + 1) * 256],
                            cs_sb[:, k * 256:(k + 1) * 256],
                            xct_sb[:, k * B + b:k * B + b + 1],
                        )

            # incremental build plan: k-tiles each t-block adds
            seen = set()
            addks = []
            for tb in range(NTB):
                new = [k for k in band[tb] if k not in seen]
                seen.update(new)
                addks.append(new)

            csbs = []
            for b in range(B):
                csb = csb_pool.tile([128, KT * 2 * FPC], f16, tag=f"csb{b}")
                build_csb(csb, b, addks[0])
                csbs.append(csb)

            # tb-outer schedule: spectrogram for all samples at this
            # t-block, then fc1 for its 256 t-chunks, so the W1 stream
            # drains continuously instead of all at the end.
            for tb in range(NTB):
                t0 = tb * TBS
                ks = band[tb]
                s_stage = stage_pool.tile([128, B * TBS], f32)
                for b in range(B):
                    csb = csbs[b]
                    ps_re = ps_pool.tile([128, TBS], f32, tag="psre")
                    ps_im = ps_pool.tile([128, TBS], f32, tag="psim")
                    for i, k in enumerate(ks):
                        blk = blkidx[(k, tb)]
                        rhs = wint_sb[:, blk * TBS:(blk + 1) * TBS]
                        nc.tensor.matmul(
                            ps_re[:], csb[:, k * 256:k * 256 + 128], rhs,
                            start=(i == 0), stop=(i == len(ks) - 1),
                        )
                        nc.tensor.matmul(
                            ps_im[:], csb[:, k * 256 + 128:(k + 1) * 256], rhs,
                            start=(i == 0), stop=(i == len(ks) - 1),
                        )
                    t_re = sq_pool.tile([128, TBS], f32, tag="tre")
                    t_im = sq_pool.tile([128, TBS], f32, tag="tim")
                    nc.scalar.activation(t_re[:], ps_re[:], Square)
                    nc.scalar.activation(t_im[:], ps_im[:], Square)
                    nc.vector.tensor_add(
                        s_stage[:, b * TBS:(b + 1) * TBS], t_re[:], t_im[:]
                    )
                    nc.vector.tensor_copy(
                        s_bf[:, b * T + t0:b * T + t0 + TBS],
                        s_stage[:, b * TBS:(b + 1) * TBS],
                    )
                    if tb + 1 < NTB:
                        build_csb(csbs[b], b, addks[tb + 1])
                s_stage_r = s_stage[:].rearrange("p (b t) -> p b t", b=B)
                s_out_r = s_out_d[:, :, t0:t0 + TBS].rearrange("b p t -> p b t")
                nc.scalar.dma_start(s_out_r[:, :B // 2], s_stage_r[:, :B // 2])
                nc.scalar.dma_start(s_out_r[:, B // 2:], s_stage_r[:, B // 2:])

                # ---- fc1 partial for this t-block's chunks ----
                # 4-way column tiling: chunks t..t+3 run concurrently in
                # separate 32-column groups of the PE array, accumulating
                # into separate PSUM banks.
                hp = [
                    hps_pool.tile([128, HID], f32, name=f"hps{j}", tag=f"hps{j}")
                    for j in range(4)
                ]
                for sl in range(TBS // GS):
                    c0 = (t0 + sl * GS) * HID
                    w1t = w1_pool.tile([128, GS * HID], f16)
                    nc.sync.dma_start(w1t[:], w1h_d[:, c0:c0 + GS * HID])
                    for g in range(0, GS, 4):
                        t = t0 + sl * GS + g
                        for j in range(4):
                            nc.tensor.matmul(
                                hp[j][32 * j:32 * j + B, :],
                                s_bf_r[:, t + j, :],
                                w1t[:, (g + j) * HID:(g + j + 1) * HID],
                                start=(t + j == t0 + j),
                                stop=(t + j == t0 + TBS - 4 + j),
                                tile_position=(0, 32 * j),
                            )
                for j in range(4):
                    nc.vector.tensor_add(
                        h_acc[:], h_acc[:], hp[j][32 * j:32 * j + B, :]
                    )
            nc.gpsimd.dma_start(h_out_d[:], h_acc[:])

    nc.compile()
    return nc


def _host_precompute(x, lambd):
    sigma = float(abs(np.float32(np.asarray(lambd))))
    n_idx = np.arange(1024, dtype=np.float64)[:, None]   # padded n (1023 -> 0)
    t_idx = np.arange(T, dtype=np.float64)[None, :]

    xc = (x - x.mean(axis=-1, keepdims=True)).astype(np.float32)  # [B, N]
    xct = np.zeros((1024, B), np.float32)
    xct[:N] = xc.T
    xct = xct.reshape(KT, 128, B).transpose(1, 0, 2).reshape(128, KT * B)
    xct = np.ascontiguousarray(xct)

    winT = np.exp(-0.5 * ((n_idx - t_idx) / sigma) ** 2).astype(np.float32)
    winT[N:] = 0.0
    wfull = winT.reshape(KT, 128, T)      # [k, p, t]

    cs_list = []
    for c in range(NCORES):
        f_idx = (c * FPC + np.arange(FPC, dtype=np.float64))[None, :]
        ang = 2.0 * np.pi * n_idx * f_idx / (2.0 * N)
        cs = np.concatenate(
            [np.cos(ang), np.sin(ang)], axis=1
        ).astype(np.float32)                                  # [1024, 256]
        cs[N:] = 0.0
        cs = cs.reshape(KT, 128, 2 * FPC).transpose(1, 0, 2)
        cs_list.append(np.ascontiguousarray(cs.reshape(128, KT * 2 * FPC)))

    return sigma, xct, wfull, cs_list


def _transpose_w1(W1):
    """Blocked transpose W1 [HID, F*T] -> [F*T, HID] in fp16."""
    feat = W1.shape[1]
    W1T = np.empty((feat, HID), np.float16)
    bs = 8192
    for i in range(0, feat, bs):
        W1T[i:i + bs] = W1[:, i:i + bs].T.astype(np.float16)
    return W1T


def kernel(x, lambd, W1, b1, W2, b2):
    global LAST_RESULT
    from concourse.bass_utils import run_bass_kernel_spmd

    x = np.asarray(x, np.float32)
    W1 = np.asarray(W1, np.float32)
    b1 = np.asarray(b1, np.float32)
    W2 = np.asarray(W2, np.float32)
    b2 = np.asarray(b2, np.float32)

    sigma, xct, wfull, cs_list = _host_precompute(x, lambd)
    band = _make_band(sigma)
    blocks, _ = _make_blocks(band)
    wint = np.concatenate(
        [wfull[k][:, tb * TBS:(tb + 1) * TBS] for (k, tb) in blocks], axis=1
    )
    wint = np.ascontiguousarray(wint.astype(np.float16))

    W1T = _transpose_w1(W1)                     # [F*T, HID] fp16
    w1h = W1T.reshape(NCORES, 128, T * HID)     # per-core contiguous views

    nc = _build_program(band, len(blocks))

    in_maps = [
        dict(xct=xct, wint=wint, cs=cs_list[c], w1h=np.ascontiguousarray(w1h[c]))
        for c in range(NCORES)
    ]
    repeat = int(os.environ.get("KERNEL_REPEAT", "1"))
    trace = bool(os.environ.get("KERNEL_TRACE"))
    times = []
    res = None
    for _ in range(repeat):
        res = run_bass_kernel_spmd(nc, in_maps, list(range(NCORES)), trace=trace)
        if res.exec_time_ns is not None:
            times.append(res.exec_time_ns)
    LAST_RESULT = res
    global LAST_TIMES
    LAST_TIMES = times

    s = np.concatenate(
        [res.results[c]["s_out"] for c in range(NCORES)], axis=1
    )                                            # [B, F, T]
    h = np.zeros((B, HID), np.float32)
    for c in range(NCORES):
        h += res.results[c]["h_out"]
    h = np.maximum(h + b1, 0.0)
    out = h @ W2.T + b2
    return out.astype(np.float32), s[:, None].astype(np.float32)
